# revision 1
# baseline (speedup 1.0000x reference)
"""Balanced BCE loss with per-sample dynamic top-k negative mining on 8 TRN2 cores.

Math: for each sample the reference computes
    pos_count = sum(gt*mask), neg_raw = sum((1-gt)*mask)
    neg_count = min(neg_raw, 3*pos_count), k = int(neg_count)
    loss = BCE(pred, gt);  pos_loss = sum(loss*positive)
    neg_topk = sum of k largest loss*negative values
    per_sample = (pos_loss + neg_topk) / (pos_count + neg_count + eps); mean over N.

Every negative position has loss > 0 (p is bounded away from {0,1}), so the
neg_loss vector has exactly neg_raw nonzero entries.  Whenever
neg_raw <= 3*pos_count, k == neg_raw and the top-k sum equals the FULL sum of
negative losses.  The device kernel therefore computes per sample
    A = sum(gt*mask)        M = sum(mask)          (B = M - A)
    C = sum(gt*mask*ln(p))  D = sum((mask-gt*mask)*ln(1-p))
(A, C, D reduced on TensorE with a ones[128,1] stationary vector; M comes for
free from the accum_out of the ScalarE pass that casts mask to bf16.)
and the host combines 16x4 scalars.  If a sample ever violates
neg_raw <= 3*pos_count, the host recomputes that sample exactly (numpy).

Device mapping: data-parallel over N, 2 samples/core.  Each [640,640] sample
is viewed as [128, 3200], processed in free-dim chunks (800/1600/800: small
first chunk so compute starts early, small last chunk for a short tail):
  - ScalarE: m16 = bf16(mask) computed as Ln((e-1)*mask + 1) (exact 0->0,
    1->1; using Ln keeps all ScalarE ops in one activation-table set, and
    accum_out yields sum(mask) for free); lp = Ln(p); l1p = Ln(1-p) via
    activation scale/bias.  All bf16 out.
  - VectorE: gm = gt*mask as one f32 tensor_tensor with bf16 output (1x);
    neg = m16-gm, t1 = gm*lp, t2 = neg*l1p as bf16 tensor_tensor (2x mode);
    plus the PSUM->SBUF copies of the accumulators.
  - TensorE: reduces gm/t1/t2 with a stationary ones[128,1] bf16 vector,
    accumulating [1,400] column sums in PSUM across each sample (A, C, D).
  - SP issues all input DMAs; output-DMA triggers are emitted after every
    input trigger so SP's in-order stream never head-of-line blocks later
    input DMAs behind a compute chain.
Host sums the [128,S*NCHUNKS] sum(mask) stats and [S,3,400] A/C/D partials
in float64.  bf16 is exact for the 0/1 tensors and products with them; only
ln values are rounded (~2^-9 relative, averaging out over ~100k summed
elements).  sum(mask) carries tiny Ln-table error, so the host rounds
neg_raw to the exact integer.
"""

import os
import sys

# defensive: if a previous process left a NeuronCore wedged, ask NRT to
# reset cores at init (read before first jax/NRT touch; harmless otherwise)
os.environ.setdefault("NEURON_RT_RESET_CORES", "1")

if "/opt/trn_rl_repo" not in sys.path:
    sys.path.insert(0, "/opt/trn_rl_repo")

import numpy as np

N, H, W = 16, 640, 640
NEG_RATIO = 3.0
EPS = 1e-8
N_CORES = 8
S = N // N_CORES          # samples per core
P = 128
FREE = H * W // P         # 3200
# per-sample free-dim chunk plans: small first chunk so compute starts
# early; the LAST sample ends with two 400-wide chunks so the final
# DMA->ln->products->matmul dependency chain is as short as possible
CHUNK_PLANS = ((800, 1600, 800), (800, 1600, 800))
NCOLS = sum(len(p) for p in CHUNK_PLANS)  # total sum(mask) stats columns
MM = 400                  # matmul sub-chunk (PSUM bank: <=512 f32)

_STATE = {}


def _build():
    import concourse.bass as bass
    import concourse.tile as tile
    from concourse import bacc, mybir

    f32 = mybir.dt.float32
    bf16 = mybir.dt.bfloat16
    Alu = mybir.AluOpType
    Act = mybir.ActivationFunctionType

    nc = bacc.Bacc("TRN2", target_bir_lowering=False, debug=False,
                   num_devices=N_CORES)
    pred_d = nc.dram_tensor("pred", [S, H, W], f32, kind="ExternalInput").ap()
    gt_d = nc.dram_tensor("gt", [S, H, W], f32, kind="ExternalInput").ap()
    mask_d = nc.dram_tensor("mask", [S, H, W], f32, kind="ExternalInput").ap()
    # per (sample, chunk): one per-partition partial sum(mask) column
    stats_d = nc.dram_tensor("stats", [P, NCOLS], f32,
                             kind="ExternalOutput").ap()
    acd_d = nc.dram_tensor("acd", [S, 3, MM], f32, kind="ExternalOutput").ap()

    with tile.TileContext(nc) as tc:
        with tc.tile_pool(name="cst", bufs=1) as cst, \
             tc.tile_pool(name="inp", bufs=6) as inp, \
             tc.tile_pool(name="mid", bufs=3) as mid, \
             tc.tile_pool(name="res", bufs=1) as res, \
             tc.tile_pool(name="ps", bufs=2, space="PSUM") as psp:
            ones = cst.tile([P, 1], bf16)
            nc.gpsimd.memset(ones[:], 1.0)
            stats = res.tile([P, NCOLS], f32)

            deferred_outs = []
            colgen = iter(range(NCOLS))

            for s in range(S):
                pred_v = pred_d[s].rearrange("(p a) w -> p (a w)", p=P)
                gt_v = gt_d[s].rearrange("(p a) w -> p (a w)", p=P)
                mask_v = mask_d[s].rearrange("(p a) w -> p (a w)", p=P)
                accA = psp.tile([1, MM], f32, tag="accA", name=f"accA_{s}")
                accC = psp.tile([1, MM], f32, tag="accC", name=f"accC_{s}")
                accD = psp.tile([1, MM], f32, tag="accD", name=f"accD_{s}")
                CHUNKS = CHUNK_PLANS[s]
                NSTEPS = sum(ch // MM for ch in CHUNKS)
                off = 0
                step = 0
                for c, CH in enumerate(CHUNKS):
                    sl = slice(off, off + CH)
                    off += CH
                    tm = inp.tile([P, CH], f32, tag="mask",
                                  name=f"tm_{s}_{c}")
                    tg = inp.tile([P, CH], f32, tag="gt", name=f"tg_{s}_{c}")
                    tp = inp.tile([P, CH], f32, tag="pred",
                                  name=f"tp_{s}_{c}")
                    nc.sync.dma_start(tm[:], mask_v[:, sl])
                    nc.sync.dma_start(tg[:], gt_v[:, sl])
                    nc.sync.dma_start(tp[:], pred_v[:, sl])

                    # bf16 "cast" of the 0/1 mask as ln((e-1)*x + 1),
                    # which is exactly 0->0, 1->1 -- using Ln keeps every
                    # ScalarE op in one activation-table set (single
                    # ACT_TABLE_LOAD); accum gives sum(mask) for free
                    m16 = mid.tile([P, CH], bf16, tag="m16",
                                   name=f"m16_{s}_{c}")
                    j = next(colgen)
                    nc.scalar.activation(m16[:], tm[:], Act.Ln,
                                         bias=1.0, scale=float(np.e - 1.0),
                                         accum_out=stats[:, j:j + 1])
                    lp = mid.tile([P, CH], bf16, tag="lp", name=f"lp_{s}_{c}")
                    nc.scalar.activation(lp[:], tp[:], Act.Ln)
                    l1p = mid.tile([P, CH], bf16, tag="l1p",
                                   name=f"l1p_{s}_{c}")
                    nc.scalar.activation(l1p[:], tp[:], Act.Ln,
                                         bias=1.0, scale=-1.0)
                    # gm = gt*mask, f32 inputs, bf16 out (1x mode)
                    gm = mid.tile([P, CH], bf16, tag="gm", name=f"gm_{s}_{c}")
                    nc.vector.tensor_tensor(gm[:], tg[:], tm[:], Alu.mult)

                    # t1 before neg: t1 needs only gm+lp, while neg waits
                    # on ScalarE's m16 -- this order never stalls t1 behind it
                    t1 = mid.tile([P, CH], bf16, tag="t1", name=f"t1_{s}_{c}")
                    nc.vector.tensor_tensor(t1[:], gm[:], lp[:], Alu.mult)
                    neg = mid.tile([P, CH], bf16, tag="neg",
                                   name=f"neg_{s}_{c}")
                    nc.vector.tensor_tensor(neg[:], m16[:], gm[:],
                                            Alu.subtract)
                    t2 = mid.tile([P, CH], bf16, tag="t2", name=f"t2_{s}_{c}")
                    nc.vector.tensor_tensor(t2[:], neg[:], l1p[:], Alu.mult)

                    for m in range(CH // MM):
                        first = step == 0
                        last = step == NSTEPS - 1
                        step += 1
                        nc.tensor.matmul(accA[:], ones[:],
                                         gm[:, bass.ts(m, MM)],
                                         start=first, stop=last)
                        nc.tensor.matmul(accC[:], ones[:],
                                         t1[:, bass.ts(m, MM)],
                                         start=first, stop=last)
                        nc.tensor.matmul(accD[:], ones[:],
                                         t2[:, bass.ts(m, MM)],
                                         start=first, stop=last)

                # copy PSUM accumulators out now, but defer the output DMA
                # triggers so SP's in-order stream never blocks later input
                # DMAs behind this sample's compute chain
                ot = res.tile([1, 3 * MM], f32, tag=f"ot_{s}",
                              name=f"ot_{s}")
                for i, acc in enumerate((accA, accC, accD)):
                    osl = ot[:, i * MM:(i + 1) * MM]
                    if s == S - 1:
                        # last sample: ScalarE is idle by now, and copies
                        # there overlap VectorE's final products
                        nc.scalar.copy(osl, acc[:])
                    else:
                        nc.vector.tensor_copy(osl, acc[:])
                deferred_outs.append((acd_d[s].rearrange("q m -> (q m)"), ot))

            for dst, ot in deferred_outs:
                nc.sync.dma_start(dst, ot[:])
            nc.sync.dma_start(stats_d[:], stats[:])
    nc.compile()
    return nc


def _get_nc():
    if "nc" not in _STATE:
        _STATE["nc"] = _build()
    return _STATE["nc"]


def _host_topk_fallback(p, g, m):
    """Exact per-sample reference semantics in numpy (rare path)."""
    p = p.astype(np.float32)
    positive = g * m
    negative = (1.0 - g) * m
    pos_count = positive.sum(dtype=np.float64)
    neg_count = min(negative.sum(dtype=np.float64), pos_count * NEG_RATIO)
    log_p = np.maximum(np.log(p), -100.0)
    log_1mp = np.maximum(np.log1p(-p), -100.0)
    loss = -(g * log_p + (1.0 - g) * log_1mp)
    pos_loss_sum = (loss * positive).sum(dtype=np.float64)
    neg_loss = (loss * negative).ravel()
    k = int(neg_count)
    if k > 0:
        top = np.partition(neg_loss, len(neg_loss) - k)[len(neg_loss) - k:]
        neg_topk = top.sum(dtype=np.float64)
    else:
        neg_topk = 0.0
    return (pos_loss_sum + neg_topk) / (pos_count + neg_count + EPS)


def _combine(results, p, g, m):
    losses = []
    for c in range(N_CORES):
        st = results[c]["stats"].astype(np.float64)  # [128, NCOLS]
        n0 = len(CHUNK_PLANS[0])
        st = np.array([st[:, :n0].sum(), st[:, n0:].sum()])  # [S] = M
        acd = results[c]["acd"].astype(np.float64).sum(axis=2)  # [S,3] A,C,D
        for s in range(S):
            M = st[s]
            A, C, D = acd[s]
            pos_count = A
            # M comes from an Ln-based cast; round to the exact integer count
            neg_raw = round(M - A)
            neg_count = min(neg_raw, pos_count * NEG_RATIO)
            k = int(neg_count)
            if k >= int(round(neg_raw)):
                # top-k covers every (strictly positive) negative loss
                losses.append((-C - D) / (pos_count + neg_count + EPS))
            else:
                i = c * S + s
                losses.append(_host_topk_fallback(p[i], g[i], m[i]))
    return np.float32(np.mean(losses))


def _in_maps(p, g, m):
    return [
        {"pred": p[c * S:(c + 1) * S],
         "gt": g[c * S:(c + 1) * S],
         "mask": m[c * S:(c + 1) * S]}
        for c in range(N_CORES)
    ]


def kernel(pred, gt, mask):
    from concourse import bass_utils

    p = np.ascontiguousarray(pred[:, 0], dtype=np.float32)   # [N,H,W]
    g = np.ascontiguousarray(gt, dtype=np.float32)
    m = np.ascontiguousarray(mask, dtype=np.float32)

    nc = _get_nc()
    in_maps = _in_maps(p, g, m)
    try:
        res = bass_utils.run_bass_kernel_spmd(nc, in_maps,
                                              core_ids=list(range(N_CORES)))
    except Exception:
        # one retry: transient device wedge from a prior process
        res = bass_utils.run_bass_kernel_spmd(nc, in_maps,
                                              core_ids=list(range(N_CORES)))
    return _combine(res.results, p, g, m)



# revision 2
# speedup vs baseline: 1.0635x; 1.0635x over previous
"""Balanced BCE loss with per-sample dynamic top-k negative mining on 8 TRN2 cores.

Math: for each sample the reference computes
    pos_count = sum(gt*mask), neg_raw = sum((1-gt)*mask)
    neg_count = min(neg_raw, 3*pos_count), k = int(neg_count)
    loss = BCE(pred, gt);  pos_loss = sum(loss*positive)
    neg_topk = sum of k largest loss*negative values
    per_sample = (pos_loss + neg_topk) / (pos_count + neg_count + eps); mean over N.

Every negative position has loss > 0 (p is bounded away from {0,1}), so
whenever neg_raw <= 3*pos_count the top-k sum equals the FULL sum of negative
losses, and the combined masked loss sum is

    pos_loss + neg_sum = -sum(mask * ln q),   q = p if gt==1 else 1-p.

Using q = |p + gt - 1| and ln|d| = 0.5*ln(d^2), the device computes per chunk
    d = (p - 1) + gt          (scalar_tensor_tensor, bf16 out)
    v = d * d                 (tensor_tensor, bf16, DVE 2x mode)
    w = Ln(v)                 (ScalarE activation, bf16; w = 2*ln q)
    t = (mask * 1) * w        (scalar_tensor_tensor, accum_out -> T column)
and the host forms T = sum(accum)/2.  pos_count and sum(mask) are exact 0/1
sums computed on the host in numpy (f64), so the fallback condition
neg_raw > 3*pos_count is exact; violating samples are recomputed exactly on
the host (never happens for random 0/1 data, kept for safety).

No TensorE/PSUM at all.  Inputs are host-packed per (core, sample) so each
chunk's DMA pulls pred|gt|mask per-partition-contiguous (12*CH bytes per
partition per trigger -> 4.8-19.2 KB DMA packets instead of 3.2 KB), data-
parallel over N: 2 samples/core, each [640,640] viewed as [128, 3200].
DVE is software-pipelined: the accumulating product of chunk c issues after
(d, v) of chunk c+1 so the DVE never stalls waiting on ScalarE's Ln.
"""

import os
import sys

# defensive: if a previous process left a NeuronCore wedged, ask NRT to
# reset cores at init (read before first jax/NRT touch; harmless otherwise)
os.environ.setdefault("NEURON_RT_RESET_CORES", "1")

if "/opt/trn_rl_repo" not in sys.path:
    sys.path.insert(0, "/opt/trn_rl_repo")

import numpy as np

N, H, W = 16, 640, 640
NEG_RATIO = 3.0
EPS = 1e-8
N_CORES = 8
S = N // N_CORES          # samples per core
P = 128
FREE = H * W // P         # 3200
# per-sample free-dim chunk plans: big head chunks for large DMA packets,
# small tail chunks on the last sample so the final dependency chain is short
CHUNK_PLANS = ((1600, 1600), (1600, 800, 400, 400))
NCHUNKS = sum(len(cp) for cp in CHUNK_PLANS)   # accum columns

_STATE = {}


def _build():
    import concourse.tile as tile
    from concourse import bacc, mybir

    f32 = mybir.dt.float32
    bf16 = mybir.dt.bfloat16
    Alu = mybir.AluOpType
    Act = mybir.ActivationFunctionType

    nc = bacc.Bacc("TRN2", target_bir_lowering=False, debug=False,
                   num_devices=N_CORES)
    # packed input: per sample, per partition: [pred CH | gt CH | mask CH]
    # repeated per chunk -- every chunk DMA is per-partition contiguous
    pk_d = nc.dram_tensor("pk", [S, P, 3 * FREE], f32,
                          kind="ExternalInput").ap()
    stats_d = nc.dram_tensor("stats", [P, NCHUNKS], f32,
                             kind="ExternalOutput").ap()

    with tile.TileContext(nc) as tc:
        with tc.tile_pool(name="inp", bufs=1) as inp, \
             tc.tile_pool(name="mid", bufs=3) as mid, \
             tc.tile_pool(name="res", bufs=1) as res:
            stats = res.tile([P, NCHUNKS], f32)

            col = 0
            pending = None  # deferred accumulating product (sw pipeline)
            for s in range(S):
                off = 0
                for c, CH in enumerate(CHUNK_PLANS[s]):
                    lo = 3 * off
                    off += CH
                    chk = inp.tile([P, 3 * CH], f32, tag=f"chk_{s}_{c}",
                                   name=f"chk_{s}_{c}")
                    nc.sync.dma_start(chk[:], pk_d[s][:, lo:lo + 3 * CH])
                    tp = chk[:, 0:CH]
                    tg = chk[:, CH:2 * CH]
                    tm = chk[:, 2 * CH:3 * CH]

                    # d = (p - 1) + gt;  |d| = q, exact-ish in f32 ALU then
                    # bf16 rounded (2^-9 rel -> ~2e-3 abs on ln q, averages out)
                    d = mid.tile([P, CH], bf16, tag="d", name=f"d_{s}_{c}")
                    nc.vector.scalar_tensor_tensor(d[:], tp, -1.0, tg,
                                                   Alu.add, Alu.add)
                    # v = d*d (2x mode); ln(v) = 2*ln q
                    v = mid.tile([P, CH], bf16, tag="v", name=f"v_{s}_{c}")
                    nc.vector.tensor_tensor(v[:], d[:], d[:], Alu.mult)
                    # w = Ln(v) on ScalarE
                    w = mid.tile([P, CH], bf16, tag="w", name=f"w_{s}_{c}")
                    nc.scalar.activation(w[:], v[:], Act.Ln)

                    # issue the previous chunk's accumulating product now so
                    # the in-order DVE queue never stalls waiting on ScalarE
                    if pending is not None:
                        _emit_t(nc, mid, pending, Alu)
                    pending = (tm, w, stats[:, col:col + 1], f"{s}_{c}")
                    col += 1
            _emit_t(nc, mid, pending, Alu)

            nc.sync.dma_start(stats_d[:], stats[:])
    nc.compile()
    return nc


def _emit_t(nc, mid, job, Alu):
    from concourse import mybir
    bf16 = mybir.dt.bfloat16
    tm, w, acol, tag = job
    CH = w.shape[1]
    t = mid.tile([P, CH], bf16, tag="t", name=f"t_{tag}")
    nc.vector.scalar_tensor_tensor(t[:], tm, 1.0, w[:], Alu.mult, Alu.mult,
                                   accum_out=acol)


def _get_nc():
    if "nc" not in _STATE:
        _STATE["nc"] = _build()
    return _STATE["nc"]


def _host_topk_fallback(p, g, m):
    """Exact per-sample reference semantics in numpy (rare path)."""
    p = p.astype(np.float32)
    positive = g * m
    negative = (1.0 - g) * m
    pos_count = positive.sum(dtype=np.float64)
    neg_count = min(negative.sum(dtype=np.float64), pos_count * NEG_RATIO)
    log_p = np.maximum(np.log(p), -100.0)
    log_1mp = np.maximum(np.log1p(-p), -100.0)
    loss = -(g * log_p + (1.0 - g) * log_1mp)
    pos_loss_sum = (loss * positive).sum(dtype=np.float64)
    neg_loss = (loss * negative).ravel()
    k = int(neg_count)
    if k > 0:
        top = np.partition(neg_loss, len(neg_loss) - k)[len(neg_loss) - k:]
        neg_topk = top.sum(dtype=np.float64)
    else:
        neg_topk = 0.0
    return (pos_loss_sum + neg_topk) / (pos_count + neg_count + EPS)


# per-sample accum column ranges (sample s covers cols [CSPLIT[s], CSPLIT[s+1]))
CSPLIT = np.cumsum([0] + [len(cp) for cp in CHUNK_PLANS]).tolist()


def _combine(results, p, g, m, A_all, M_all):
    losses = []
    for c in range(N_CORES):
        st = results[c]["stats"].astype(np.float64)  # [128, NCHUNKS]
        for s in range(S):
            i = c * S + s
            A = A_all[i]
            neg_raw = M_all[i] - A
            neg_count = min(neg_raw, A * NEG_RATIO)
            if int(neg_count) >= int(neg_raw):
                # top-k covers every (strictly positive) negative loss;
                # T = sum(mask*2*ln q) -> loss sum = -T/2
                T = st[:, CSPLIT[s]:CSPLIT[s + 1]].sum()
                losses.append((-0.5 * T) / (A + neg_count + EPS))
            else:
                losses.append(_host_topk_fallback(p[i], g[i], m[i]))
    return np.float32(np.mean(losses))


def _pack(p, g, m):
    """Interleave pred|gt|mask per chunk, per partition: [N, P, 3*FREE]."""
    pk = np.empty((N, P, 3 * FREE), dtype=np.float32)
    pr = p.reshape(N, P, FREE)
    gr = g.reshape(N, P, FREE)
    mr = m.reshape(N, P, FREE)
    for plan_s, cols in ((0, CHUNK_PLANS[0]), (1, CHUNK_PLANS[1])):
        pass
    # chunk layout is identical for every sample index modulo its plan; but
    # plans differ per sample-slot (s within core), i.e. global sample i has
    # plan CHUNK_PLANS[i % S].  Pack each global sample with its slot's plan.
    for i in range(N):
        plan = CHUNK_PLANS[i % S]
        off = 0
        for CH in plan:
            lo = 3 * off
            pk[i, :, lo:lo + CH] = pr[i, :, off:off + CH]
            pk[i, :, lo + CH:lo + 2 * CH] = gr[i, :, off:off + CH]
            pk[i, :, lo + 2 * CH:lo + 3 * CH] = mr[i, :, off:off + CH]
            off += CH
    return pk


def _in_maps(pk):
    return [{"pk": pk[c * S:(c + 1) * S]} for c in range(N_CORES)]


def kernel(pred, gt, mask):
    from concourse import bass_utils

    p = np.ascontiguousarray(pred[:, 0], dtype=np.float32)   # [N,H,W]
    g = np.ascontiguousarray(gt, dtype=np.float32)
    m = np.ascontiguousarray(mask, dtype=np.float32)

    # exact 0/1 counts on host (cheap, removes all device rounding concerns
    # from the fallback condition)
    M_all = m.sum(axis=(1, 2), dtype=np.float64)             # [N]
    A_all = (g * m).sum(axis=(1, 2), dtype=np.float64)       # [N]

    pk = _pack(p, g, m)
    nc = _get_nc()
    in_maps = _in_maps(pk)
    try:
        res = bass_utils.run_bass_kernel_spmd(nc, in_maps,
                                              core_ids=list(range(N_CORES)))
    except Exception:
        # one retry: transient device wedge from a prior process
        res = bass_utils.run_bass_kernel_spmd(nc, in_maps,
                                              core_ids=list(range(N_CORES)))
    return _combine(res.results, p, g, m, A_all, M_all)
